# revision 1
# baseline (speedup 1.0000x reference)
"""CstLoss on Trainium2 — self-contained Bass/Tile SPMD kernel (8 NeuronCores).

Reference math (per [N=64, C=17, H=128, W=128] f32 pair output/target):
  h/w marginal means of each map -> softmax over the 128-axis -> l2
  normalize -> sim_pos = mean of matched-channel cosines, sim = sum of
  mean-over-batch all-pairs cosines, loss = -log(sim_pos/sim)/C/N.

Key algebra used here:
  * softmax denominator cancels under l2 normalization (and the reference's
    1e-8 norm clamp never binds since max(exp) = 1), so each projection only
    needs q = e / ||e||_2 with e = exp((S - max S)/W), S = raw row/col sums.
  * sum_ij dot(qo_i, qt_j) = dot(sum_i qo_i, sum_j qt_j), so the CxC pair
    matrix is never materialized: per n we only need channel sums U, V.

Sharding: data-parallel over the batch axis, 8 entries per core. Each core
returns per-map matched dots ("pos" [136]) and channel sums ("u"/"v"
[8, 256]); the host reduces those to the scalar loss (the two "all-reduce a
scalar" steps of the hint, done on host).

Per-core kernel: 136 maps per tensor = 128-map main batch in natural layout
(one full 64KB map per SBUF partition -> large contiguous DMA descriptors)
+ 8-map tail in 2D layout (h on partitions). h-projections: DVE segmented
reduces. w-projections: PE transpose-accumulate of h-slices into PSUM
(exact: transposes move data, PSUM accumulates in f32). Tail w-projections:
ones-vector matmuls + SBUF scatter DMA.
"""

import contextlib
import ctypes
import sys
import types
from contextlib import ExitStack

import numpy as np

import concourse.bacc as bacc
import concourse.tile as tile
from concourse import mybir
from concourse.bass_utils import run_bass_kernel_spmd

F32 = mybir.dt.float32
AX = mybir.AxisListType
ALU = mybir.AluOpType
ACT = mybir.ActivationFunctionType

N, C, H, W = 64, 17, 128, 128
NCORES = 8
NLOC = N // NCORES           # 8 batch entries per core
MAPS = NLOC * C              # 136 maps per tensor per core
MAIN = 128                   # maps in the main batch
TAIL = MAPS - MAIN           # 8 maps in the tail
HCHUNK = 32                  # h-rows per main sub-chunk
NCHUNKS = H // HCHUNK


def _install_ntff_hook():
    """Provide antenv.axon_hooks if the image lacks it (needed only when
    run_bass_kernel_spmd is called with trace=True; harmless otherwise)."""
    if "antenv.axon_hooks" in sys.modules:
        return
    so_path = "/opt/axon/libaxon_pjrt.so"
    hook = None
    try:
        lib = ctypes.CDLL(so_path)
        if hasattr(lib, "axon_start_nrt_profile"):
            lib.axon_start_nrt_profile.argtypes = [
                ctypes.POINTER(ctypes.c_int64),
                ctypes.c_size_t,
            ]
            lib.axon_start_nrt_profile.restype = ctypes.c_int64
            lib.axon_stop_nrt_profile.argtypes = [ctypes.c_char_p]
            lib.axon_stop_nrt_profile.restype = ctypes.c_int64

            @contextlib.contextmanager
            def _hook(output_dir, device_ids):
                import jax

                jax.devices()
                if device_ids:
                    ids = (ctypes.c_int64 * len(device_ids))(*device_ids)
                    rc = lib.axon_start_nrt_profile(ids, len(device_ids))
                else:
                    rc = lib.axon_start_nrt_profile(None, 0)
                if rc != 0:
                    raise RuntimeError(f"axon_start_nrt_profile rc={rc}")
                try:
                    yield
                finally:
                    n = lib.axon_stop_nrt_profile(str(output_dir).encode())
                    print(f"profile: {n} file(s) in {output_dir}", file=sys.stderr)

            hook = _hook
    except OSError:
        pass
    mod = types.ModuleType("antenv.axon_hooks")
    mod.get_axon_ntff_profile_hook = lambda: hook
    mod.set_axon_ntff_profile_hook = lambda h: None
    sys.modules["antenv.axon_hooks"] = mod


_install_ntff_hook()


def _normalize(nc, workp, ap, Pn, pref):
    """In place per 128-segment: e = exp((S - max S)/W); q = e/||e||_2."""
    v = ap.rearrange("p (s w) -> p s w", w=W)
    mx = workp.tile([Pn, 2], F32, tag=f"mx{Pn}", name=f"mx_{pref}")
    nc.vector.reduce_max(mx[:], v, axis=AX.X)
    nb = workp.tile([Pn, 2], F32, tag=f"nb{Pn}", name=f"nb_{pref}")
    nc.scalar.mul(nb[:], mx[:], -1.0 / W)
    ssq = workp.tile([Pn, 2], F32, tag=f"ssq{Pn}", name=f"ssq_{pref}")
    dump = workp.tile([Pn, W], F32, tag=f"dump{Pn}", name=f"dump_{pref}")
    for s in range(2):
        nc.scalar.activation(
            v[:, s, :], v[:, s, :], ACT.Exp, bias=nb[:, s : s + 1], scale=1.0 / W
        )
        # sum of squares: ACT Square + accum_out (tensor_tensor_reduce
        # wedges the device on this runtime)
        nc.scalar.activation(dump[:], v[:, s, :], ACT.Square, accum_out=ssq[:, s : s + 1])
    sq = workp.tile([Pn, 2], F32, tag=f"sq{Pn}", name=f"sq_{pref}")
    nc.scalar.sqrt(sq[:], ssq[:])
    rn = workp.tile([Pn, 2], F32, tag=f"rn{Pn}", name=f"rn_{pref}")
    nc.vector.reciprocal(rn[:], sq[:])
    for s in range(2):
        nc.vector.tensor_scalar_mul(v[:, s, :], v[:, s, :], rn[:, s : s + 1])


def _body(tc, o_d, t_d, id_d, g0_d, gt_d, on_d, pos_d, u_d, v_d):
    nc = tc.nc
    with ExitStack() as ctx:
        consts = ctx.enter_context(tc.tile_pool(name="consts", bufs=1))
        chunks = ctx.enter_context(tc.tile_pool(name="chunks", bufs=8))
        projp = ctx.enter_context(tc.tile_pool(name="projp", bufs=1))
        tailp = ctx.enter_context(tc.tile_pool(name="tailp", bufs=1))
        workp = ctx.enter_context(tc.tile_pool(name="workp", bufs=2))
        outp = ctx.enter_context(tc.tile_pool(name="outp", bufs=1))
        # PSUM: 8 distinct tiles = 8 banks, no slot rotation (slot reuse
        # with concurrent PE traffic wedges the device: NRT status 101).
        accps = ctx.enter_context(tc.tile_pool(name="accps", bufs=1, space="PSUM"))

        ident = consts.tile([128, 128], F32)
        nc.gpsimd.dma_start(ident[:], id_d)
        g0 = consts.tile([128, NLOC], F32)
        nc.gpsimd.dma_start(g0[:], g0_d)
        gt = consts.tile([TAIL, NLOC], F32)
        nc.gpsimd.dma_start(gt[:], gt_d)
        ones = consts.tile([128, 1], F32)
        nc.gpsimd.dma_start(ones[:], on_d)

        proj_o = projp.tile([128, 2 * W], F32)
        proj_t = projp.tile([128, 2 * W], F32)
        wt_o = accps.tile([128, 128], F32)
        wt_t = accps.tile([128, 128], F32)
        U = accps.tile([NLOC, 2 * W], F32)
        Vt = accps.tile([NLOC, 2 * W], F32)

        # ---- main batches: 128 maps, one full map per partition ----
        for ti, (x_d, proj, wt) in enumerate(
            ((o_d, proj_o, wt_o), (t_d, proj_t, wt_t))
        ):
            for c in range(NCHUNKS):
                chunk = chunks.tile(
                    [128, HCHUNK * W], F32, tag="chunk", name=f"chunk{ti}_{c}"
                )
                nc.sync.dma_start(
                    chunk[:], x_d[0:MAIN, c * HCHUNK : (c + 1) * HCHUNK, :]
                )
                cv = chunk.rearrange("p (h w) -> p h w", w=W)
                nc.vector.reduce_sum(
                    proj[:, c * HCHUNK : (c + 1) * HCHUNK], cv, axis=AX.X
                )
                for j in range(HCHUNK):
                    nc.tensor.matmul(
                        wt[:],
                        cv[:, j, :],
                        ident[:],
                        is_transpose=True,
                        start=(c == 0 and j == 0),
                        stop=(c == NCHUNKS - 1 and j == HCHUNK - 1),
                    )
            wts = workp.tile([128, 128], F32, tag="wts", name=f"wts{ti}")
            nc.scalar.copy(wts[:], wt[:])
            wb = accps.tile([128, 128], F32, name=f"wb{ti}")
            nc.tensor.matmul(wb[:], wts[:], ident[:], is_transpose=True)
            nc.scalar.copy(proj[:, W : 2 * W], wb[:])

        # ---- tail: 8 maps x 2 tensors, h on partitions ----
        tail2d = tailp.tile([128, 2 * TAIL * W], F32)
        tv = tail2d.rearrange("p (m w) -> p m w", w=W)
        nc.sync.dma_start(tv[:, 0:TAIL, :], o_d[MAIN:MAPS].rearrange("m h w -> h m w"))
        nc.sync.dma_start(
            tv[:, TAIL : 2 * TAIL, :], t_d[MAIN:MAPS].rearrange("m h w -> h m w")
        )
        R = tailp.tile([128, 2 * TAIL], F32)
        nc.vector.reduce_sum(R[:], tv, axis=AX.X)
        To = tailp.tile([TAIL, 2 * W], F32)
        Tt = tailp.tile([TAIL, 2 * W], F32)
        # One PSUM bank per tensor's tail: the R-transpose at base partition
        # 0 plus two ones-matmul [1,512] chunks at bases 32/64 (matmul PSUM
        # outputs may only start at partitions 0/32/64).
        tlA = accps.tile([65, 512], F32)
        tlB = accps.tile([65, 512], F32)
        for i, (T, tl) in enumerate(((To, tlA), (Tt, tlB))):
            nc.tensor.matmul(
                tl[0:TAIL, 0:128],
                R[:, i * TAIL : (i + 1) * TAIL],
                ident[:],
                is_transpose=True,
                skip_group_check=True,
            )
            nc.scalar.copy(T[:, 0:W], tl[0:TAIL, 0:128])
            for k in range(2):
                kk = 2 * i + k
                nc.tensor.matmul(
                    tl[32 * (k + 1) : 32 * (k + 1) + 1, :],
                    ones[:],
                    tail2d[:, kk * 512 : (kk + 1) * 512],
                    skip_group_check=True,
                )
        srowA = tailp.tile([65, 512], F32)
        srowB = tailp.tile([65, 512], F32)
        for srow, tl in ((srowA, tlA), (srowB, tlB)):
            nc.scalar.copy(srow[32:33, :], tl[32:33, :])
            nc.scalar.copy(srow[64:65, :], tl[64:65, :])
        nc.gpsimd.dma_start(To[0:4, W : 2 * W], srowA[32:33, :])
        nc.gpsimd.dma_start(To[4:TAIL, W : 2 * W], srowA[64:65, :])
        nc.gpsimd.dma_start(Tt[0:4, W : 2 * W], srowB[32:33, :])
        nc.gpsimd.dma_start(Tt[4:TAIL, W : 2 * W], srowB[64:65, :])

        # ---- softmax + l2 normalize ----
        _normalize(nc, workp, proj_o[:], 128, "po")
        _normalize(nc, workp, proj_t[:], 128, "pt")
        _normalize(nc, workp, To[:], TAIL, "to")
        _normalize(nc, workp, Tt[:], TAIL, "tt")

        # ---- matched dots and per-n channel sums ----
        pos0 = outp.tile([MAIN, 1], F32)
        dumpP = workp.tile([128, 2 * W], F32, tag="dumpP")
        nc.vector.tensor_mul(dumpP[:], proj_o[:], proj_t[:])
        nc.vector.reduce_sum(pos0[:], dumpP[:], axis=AX.X)
        post = outp.tile([TAIL, 1], F32)
        dumpT = workp.tile([TAIL, 2 * W], F32, tag="dumpT")
        nc.vector.tensor_mul(dumpT[:], To[:], Tt[:])
        nc.vector.reduce_sum(post[:], dumpT[:], axis=AX.X)
        nc.tensor.matmul(U[:], g0[:], proj_o[:], start=True, stop=False)
        nc.tensor.matmul(U[:], gt[:], To[:], start=False, stop=True)
        nc.tensor.matmul(Vt[:], g0[:], proj_t[:], start=True, stop=False)
        nc.tensor.matmul(Vt[:], gt[:], Tt[:], start=False, stop=True)
        us = outp.tile([NLOC, 2 * W], F32)
        nc.scalar.copy(us[:], U[:])
        vs = outp.tile([NLOC, 2 * W], F32)
        nc.scalar.copy(vs[:], Vt[:])
        nc.sync.dma_start(u_d, us[:])
        nc.sync.dma_start(v_d, vs[:])
        nc.sync.dma_start(pos_d[0:MAIN, :], pos0[:])
        nc.sync.dma_start(pos_d[MAIN:MAPS, :], post[:])


def _build_nc():
    nc = bacc.Bacc("TRN2", target_bir_lowering=False, debug=False)
    o_d = nc.dram_tensor("o", [MAPS, H, W], F32, kind="ExternalInput").ap()
    t_d = nc.dram_tensor("t", [MAPS, H, W], F32, kind="ExternalInput").ap()
    id_d = nc.dram_tensor("ident", [128, 128], F32, kind="ExternalInput").ap()
    g0_d = nc.dram_tensor("g0", [128, NLOC], F32, kind="ExternalInput").ap()
    gt_d = nc.dram_tensor("gt", [TAIL, NLOC], F32, kind="ExternalInput").ap()
    on_d = nc.dram_tensor("ones", [128, 1], F32, kind="ExternalInput").ap()
    pos_d = nc.dram_tensor("pos", [MAPS, 1], F32, kind="ExternalOutput").ap()
    u_d = nc.dram_tensor("u", [NLOC, 2 * W], F32, kind="ExternalOutput").ap()
    v_d = nc.dram_tensor("v", [NLOC, 2 * W], F32, kind="ExternalOutput").ap()
    with tile.TileContext(nc) as tc:
        _body(tc, o_d, t_d, id_d, g0_d, gt_d, on_d, pos_d, u_d, v_d)
    nc.compile()
    return nc


_NC = None


def _get_nc():
    global _NC
    if _NC is None:
        _NC = _build_nc()
    return _NC


_IDENT = np.eye(128, dtype=np.float32)
_G0 = np.zeros((128, NLOC), np.float32)
_G0[np.arange(128), np.arange(128) // C] = 1.0
_GT = np.zeros((TAIL, NLOC), np.float32)
_GT[:, NLOC - 1] = 1.0
_ONES = np.ones((128, 1), np.float32)


def _make_in_maps(output, target):
    in_maps = []
    for i in range(NCORES):
        o = np.ascontiguousarray(output[i * NLOC : (i + 1) * NLOC]).reshape(MAPS, H, W)
        t = np.ascontiguousarray(target[i * NLOC : (i + 1) * NLOC]).reshape(MAPS, H, W)
        in_maps.append(
            {"o": o, "t": t, "ident": _IDENT, "g0": _G0, "gt": _GT, "ones": _ONES}
        )
    return in_maps


def _finish(results):
    A = 0.0
    B = 0.0
    for r in results:
        A += float(r["pos"].astype(np.float64).sum())
        B += float((r["u"].astype(np.float64) * r["v"].astype(np.float64)).sum())
    # sim_pos = 0.5*A/(N*C); sim = 0.5*B/N; loss = -log(sim_pos/sim)/(C*N)
    loss = -np.log(A / (C * B)) / (C * N)
    return np.float32(loss)


def kernel(output, target):
    output = np.asarray(output, dtype=np.float32)
    target = np.asarray(target, dtype=np.float32)
    nc = _get_nc()
    res = run_bass_kernel_spmd(nc, _make_in_maps(output, target), list(range(NCORES)))
    return _finish(res.results)


def profile(output, target):
    """Run once with NTFF tracing; returns max per-core HW exec time in ns."""
    output = np.asarray(output, dtype=np.float32)
    target = np.asarray(target, dtype=np.float32)
    nc = _get_nc()
    res = run_bass_kernel_spmd(
        nc, _make_in_maps(output, target), list(range(NCORES)), trace=True
    )
    return res.exec_time_ns

